# revision 26
# baseline (speedup 1.0000x reference)
"""Trainium2 Bass kernel for the L1Writer scatter-memory problem.

Computes   out = 0.95 * memory + einsum('bs,bshk,bshv->hkv', rho, keys, values)

Strategy: data-parallel over the flattened (B*S)=16384 token axis, 2048 rows
per core.  Each core computes its partial delta
    delta_h = K_h^T diag(rho) V_h        (per head h, contraction over 2048)
as a chain of 128-row PE matmuls accumulating in PSUM.  The 8 partial
(H,Dk,Dv) deltas are summed on the host (tiny: 256 KB each) and added to
decay*memory there.

Measured pipeline (per core, memory-bound: 16 MB of f32 K/V):
  - ~8 us fixed NEFF startup (entry barrier + engine iram loads), first
    K chunk lands ~11 us, stream runs at ~400 GB/s until ~52 us.
  - K/V interleave per 512 KB chunk (128 partitions x 4 KB contiguous
    descriptors) on the SP (sync) HWDGE queue.  rho rides first,
    zero-padded to 512 B per partition (64 B descriptors took ~6 us).
  - DVE scales keys by rho (per-partition tensor_scalar, f32 -> bf16);
    ACT copy-casts values f32 -> bf16 (and warms its activation table at
    t~0: a lazy table load + drain costs ~3 us if paid in the tail).
    Each stage is ~0.9 us per chunk against the 2.56 us DMA metronome.
  - PE runs bf16 matmuls at ~82 ns/issue: ~21 us total, comfortably
    inside the stream (f32 matmuls are 42 us and become the critical
    path end-to-end).
  - 16 heads accumulate into 2 PSUM banks ([64, 512] each, 8 heads per
    bank).  The first matmul touching each bank uses start=True: it
    clears the bank's has_written bits, so every later start=False matmul
    overwrites its region on first touch and accumulates after -- no
    zero-fill needed and no stale state across NEFF reruns.
  - Tail: PE drains (PSUM writes land ~175 ns after commit), DVE
    evacuates PSUM -> SBUF, and the 256 KB out DMA is split across the
    sync and ACT HWDGE queues (64-partition DMAs run at half rate).

Hard-won hazard notes (HW-measured, this container):
  - One DMA per semaphore threshold.  N DMAs bumping one semaphore by 16
    each with consumers waiting partial thresholds (16*(j+1)) is UNSOUND:
    the 16 SDMA engines complete their per-DMA slices with skew, so
    engines running ahead on later DMAs push the count past 16*(j+1)
    while a laggard is still landing chunk j -> torn reads, sticky
    per-NEFF-load nondeterministic corruption.
  - A DVE memset does not touch PSUM has_written bits and engine start is
    skewed by iram loads, so memset-then-accumulate races the first
    matmuls.  start=True on each bank's first matmul replaces it safely.
  - Engine drains (fusable=False) before the incs that release the PSUM
    evacuation and the final out-DMA: those consumers read within ns of
    the inc and the producer's writes land after instruction commit.
"""

import numpy as np

DECAY = 0.95
B, S, H, Dk, Dv = 4, 4096, 16, 64, 64
N_CORES = 8
NS = (B * S) // N_CORES          # 2048 rows per core
P = 128                          # partitions
CHUNKS = NS // P                 # 16 contraction chunks of 128 rows
MEGA = 4                         # chunks per buffer slot group
N_MEGA = CHUNKS // MEGA          # 4 slot groups, double-buffered
FD = H * Dk                      # 1024 features per row

_nc_cache = None


def _build_nc():
    from contextlib import ExitStack

    import concourse.bass as bass
    import concourse.mybir as mybir

    f32 = mybir.dt.float32
    bf16 = mybir.dt.bfloat16
    nc = bass.Bass()

    keys_d = nc.dram_tensor("keys", (NS, FD), f32, kind="ExternalInput")
    vals_d = nc.dram_tensor("values", (NS, FD), f32, kind="ExternalInput")
    rho_d = nc.dram_tensor("rho", (P, P), f32, kind="ExternalInput")
    out_d = nc.dram_tensor("delta", (Dk, H * Dv), f32, kind="ExternalOutput")

    # Raw bass (no Tile): this container's walrus rejects engine
    # instructions carrying >1 attached semaphore wait, so all waits are
    # standalone sequencer wait_ge ops and every hazard is hand-managed.
    #
    # Chunk c = m*4+j covers token rows [c*128, (c+1)*128).  kt/vt/ktb/vtb
    # are double-buffered over m (slot m%2), 4 chunks per slot.
    #
    # Semaphore roles:
    #   kcs[c]/vcs[c]: one per chunk DMA, +16 at completion (sound: one
    #     DMA per threshold, see module docstring)
    #   scale_sem: +1 per DVE key-scale  -> c+1 after scale of chunk c
    #   act_sem:   +1 per ACT value-cast -> c+1 after cast of chunk c
    #   pe_sem:    +1 per matmul group   -> c+1 after group of chunk c
    #   evac_sem:  2 after DVE evacuates PSUM to out_t
    #   out_sem:   +16 per out-DMA half (sync + scalar queues) -> 32 done
    with ExitStack() as ctx:
        kt = [
            ctx.enter_context(nc.sbuf_tensor(f"kt{i}", [P, MEGA, FD], f32))
            for i in range(2)
        ]
        vt = [
            ctx.enter_context(nc.sbuf_tensor(f"vt{i}", [P, MEGA, FD], f32))
            for i in range(2)
        ]
        ktb = [
            ctx.enter_context(nc.sbuf_tensor(f"ktb{i}", [P, MEGA, FD], bf16))
            for i in range(2)
        ]
        vtb = [
            ctx.enter_context(nc.sbuf_tensor(f"vtb{i}", [P, MEGA, FD], bf16))
            for i in range(2)
        ]
        rho_t = ctx.enter_context(nc.sbuf_tensor("rho_t", [P, P], f32))
        warm = ctx.enter_context(nc.sbuf_tensor("warm", [Dk, 8], f32))
        out_t = ctx.enter_context(nc.sbuf_tensor("out_t", [Dk, H * Dv], f32))
        acc = [
            ctx.enter_context(nc.psum_tensor(f"acc{i}", [Dk, 8 * Dv], f32))
            for i in range(2)
        ]
        rs = ctx.enter_context(nc.semaphore(name="rs"))
        kcs = [ctx.enter_context(nc.semaphore(name=f"kc{i}")) for i in range(CHUNKS)]
        vcs = [ctx.enter_context(nc.semaphore(name=f"vc{i}")) for i in range(CHUNKS)]
        scale_sem = ctx.enter_context(nc.semaphore(name="scale_sem"))
        act_sem = ctx.enter_context(nc.semaphore(name="act_sem"))
        evac_sem = ctx.enter_context(nc.semaphore(name="evac_sem"))
        out_sem = ctx.enter_context(nc.semaphore(name="out_sem"))
        done_sem = ctx.enter_context(nc.semaphore(name="done_sem"))
        pe_sem = ctx.enter_context(nc.semaphore(name="pe_sem"))
        block = ctx.enter_context(nc.Block())

        def chunk_rows(m, j):
            c = m * MEGA + j
            return slice(c * P, (c + 1) * P)

        @block.sync
        def _(sync):
            sync.dma_start(rho_t[:], rho_d[:]).then_inc(rs, 16)
            for m in range(N_MEGA):
                for j in range(MEGA):
                    c = m * MEGA + j
                    if m >= 2:
                        # kt slot WAR: scale of chunk c-8 consumed the slot
                        sync.wait_ge(scale_sem, (m - 2) * MEGA + j + 1)
                    sync.dma_start(
                        kt[m % 2][:, j, :], keys_d[chunk_rows(m, j), :]
                    ).then_inc(kcs[c], 16)
                    if m >= 2:
                        # vt slot WAR: cast of chunk c-8 consumed the slot
                        sync.wait_ge(act_sem, (m - 2) * MEGA + j + 1)
                    sync.dma_start(
                        vt[m % 2][:, j, :], vals_d[chunk_rows(m, j), :]
                    ).then_inc(vcs[c], 16)
            sync.wait_ge(evac_sem, 1)
            sync.dma_start(out_d[:, : H * Dv // 2], out_t[:, : H * Dv // 2]).then_inc(
                out_sem, 16
            )
            sync.wait_ge(out_sem, 32)
            # all incs that will ever fire have fired: clear every
            # semaphore so the kernel is safe to run repeatedly
            # (semaphores persist across NEFF executions)
            for s in [
                rs, *kcs, *vcs, scale_sem, act_sem, evac_sem, pe_sem, out_sem,
            ]:
                sync.sem_clear(s)

        @block.vector
        def _(vector):
            # key scales by rho, f32 -> bf16
            vector.wait_ge(rs, 16)
            for m in range(N_MEGA):
                for j in range(MEGA):
                    c = m * MEGA + j
                    vector.wait_ge(kcs[c], 16)
                    if m >= 2:
                        # ktb slot WAR: matmul group of chunk c-8 done
                        vector.wait_ge(pe_sem, (m - 2) * MEGA + j + 1)
                    vector.tensor_scalar_mul(
                        ktb[m % 2][:, j, :],
                        kt[m % 2][:, j, :],
                        rho_t[:, c : c + 1],
                    ).then_inc(scale_sem, 1)
            # PSUM bank-0 evacuation (bank 1 runs in parallel on ACT)
            vector.wait_ge(pe_sem, 16)
            vector.tensor_copy(out_t[:, :512], acc[0][:])
            # drain before inc: the out DMA reads out_t right at the edge
            vector.drain(fusable=False).then_inc(evac_sem, 1)

        @block.scalar
        def _(scalar):
            # warm the activation table immediately: a lazy load at first
            # use costs ~3 us (load + auto drain) in whatever stage pays it
            scalar.copy(warm[:], warm[:])
            # value casts f32 -> bf16
            for m in range(N_MEGA):
                for j in range(MEGA):
                    c = m * MEGA + j
                    scalar.wait_ge(vcs[c], 16)
                    if m >= 2:
                        # vtb slot WAR: matmul group of chunk c-8 done
                        scalar.wait_ge(pe_sem, (m - 2) * MEGA + j + 1)
                    scalar.copy(vtb[m % 2][:, j, :], vt[m % 2][:, j, :]).then_inc(
                        act_sem, 1
                    )
            # PSUM bank-1 evacuation + its out-DMA half, all in ACT program
            # order (no cross-engine handoff needed)
            scalar.wait_ge(pe_sem, 16)
            scalar.copy(out_t[:, 512:], acc[1][:])
            scalar.drain(fusable=False)
            scalar.dma_start(
                out_d[:, H * Dv // 2 :], out_t[:, H * Dv // 2 :]
            ).then_inc(out_sem, 16)

        @block.tensor
        def _(tensor):
            for m in range(N_MEGA):
                for j in range(MEGA):
                    c = m * MEGA + j
                    tensor.wait_ge(scale_sem, c + 1)
                    tensor.wait_ge(act_sem, c + 1)
                    first = c == 0
                    last = c == CHUNKS - 1
                    for h in range(H):
                        g, hh = divmod(h, 8)
                        mm = tensor.matmul(
                            acc[g][:, hh * Dv : (hh + 1) * Dv],
                            ktb[m % 2][:, j, h * Dk : (h + 1) * Dk],
                            vtb[m % 2][:, j, h * Dv : (h + 1) * Dv],
                            # first touch of each bank clears its
                            # has_written bits; later matmuls overwrite
                            # untouched regions and accumulate touched ones
                            start=(first and hh == 0),
                            stop=last,
                            skip_group_check=True,
                        )
                        if h == H - 1 and not last:
                            # commit-attached: only WAR consumers (slot
                            # reuse) key off these counts, and commit means
                            # the matmul's SBUF reads are done.
                            mm.then_inc(pe_sem, 1)
                    if last:
                        # the 16th inc gates the PSUM evacuation: drain so
                        # the systolic array has written PSUM before DVE
                        # reads it.
                        tensor.drain(fusable=False).then_inc(pe_sem, 1)

    return nc


def _get_nc():
    global _nc_cache
    if _nc_cache is None:
        _nc_cache = _build_nc()
    return _nc_cache


def _make_in_maps(keys, values, write_strengths):
    kf = np.ascontiguousarray(keys.reshape(B * S, FD))
    vf = np.ascontiguousarray(values.reshape(B * S, FD))
    wf = np.asarray(write_strengths).reshape(B * S)
    in_maps = []
    for c in range(N_CORES):
        sl = slice(c * NS, (c + 1) * NS)
        rho_pad = np.zeros((P, P), np.float32)
        rho_pad[:, :CHUNKS] = wf[sl].reshape(CHUNKS, P).T
        in_maps.append(
            {
                "keys": np.ascontiguousarray(kf[sl]),
                "values": np.ascontiguousarray(vf[sl]),
                "rho": rho_pad,
            }
        )
    return in_maps


def _run(in_maps, **kwargs):
    from concourse.bass_utils import run_bass_kernel_spmd

    nc = _get_nc()
    return run_bass_kernel_spmd(nc, in_maps, core_ids=list(range(N_CORES)), **kwargs)


def _assemble(memory, results):
    parts = np.stack([r["delta"] for r in results], axis=0)  # (8, 64, 1024)
    delta = parts.sum(axis=0, dtype=np.float64)  # (64, 1024) in [k, h*64+v]
    delta_hkv = delta.reshape(Dk, H, Dv).transpose(1, 0, 2)  # (H, Dk, Dv)
    out = DECAY * np.asarray(memory, dtype=np.float64) + delta_hkv
    return out.astype(np.float32)


def kernel(memory, keys, values, write_strengths):
    memory = np.asarray(memory, dtype=np.float32)
    keys = np.asarray(keys, dtype=np.float32)
    values = np.asarray(values, dtype=np.float32)
    write_strengths = np.asarray(write_strengths, dtype=np.float32)

    in_maps = _make_in_maps(keys, values, write_strengths)
    res = _run(in_maps)
    return _assemble(memory, res.results)


if __name__ == "__main__":
    rng = np.random.default_rng(0)
    mem = rng.standard_normal((H, Dk, Dv), dtype=np.float32)
    k = rng.standard_normal((B, S, H, Dk), dtype=np.float32)
    v = rng.standard_normal((B, S, H, Dv), dtype=np.float32)
    w = rng.random((B, S), dtype=np.float32)
    out = kernel(mem, k, v, w)
    ref = DECAY * mem + np.einsum(
        "bs,bshk,bshv->hkv", w.astype(np.float64), k.astype(np.float64), v.astype(np.float64)
    )
    err = np.abs(out - ref).max() / np.abs(ref).max()
    print("self-check rel err:", err)


# revision 27
# speedup vs baseline: 1.0660x; 1.0660x over previous
"""Trainium2 Bass kernel for the L1Writer scatter-memory problem.

Computes   out = 0.95 * memory + einsum('bs,bshk,bshv->hkv', rho, keys, values)

Strategy: data-parallel over the flattened (B*S)=16384 token axis, 2048 rows
per core.  Each core computes its partial delta
    delta_h = K_h^T diag(rho) V_h        (per head h, contraction over 2048)
as a chain of 128-row PE matmuls accumulating in PSUM.  The 8 partial
(H,Dk,Dv) deltas are summed on the host (tiny: 256 KB each) and added to
decay*memory there.

Measured pipeline (per core, memory-bound: 16 MB of f32 K/V):
  - ~8 us fixed NEFF startup (entry barrier + engine iram loads), first
    K chunk lands ~11 us, stream runs at ~400 GB/s until ~52 us.
  - K/V interleave per 512 KB chunk (128 partitions x 4 KB contiguous
    descriptors) on the SP (sync) HWDGE queue.  rho rides first,
    zero-padded to 512 B per partition (64 B descriptors took ~6 us).
  - DVE scales keys by rho (per-partition tensor_scalar, f32 -> bf16);
    ACT copy-casts values f32 -> bf16 (and warms its activation table at
    t~0: a lazy table load + drain costs ~3 us if paid in the tail).
    Each stage is ~0.9 us per chunk against the 2.56 us DMA metronome.
  - PE runs bf16 matmuls at ~82 ns/issue: ~21 us total, comfortably
    inside the stream (f32 matmuls are 42 us and become the critical
    path end-to-end).
  - 16 heads accumulate into 2 PSUM banks ([64, 512] each, 8 heads per
    bank).  The first matmul touching each bank uses start=True: it
    clears the bank's has_written bits, so every later start=False matmul
    overwrites its region on first touch and accumulates after -- no
    zero-fill needed and no stale state across NEFF reruns.
  - Tail: PE drains (PSUM writes land ~175 ns after commit), DVE
    evacuates PSUM -> SBUF, and the 256 KB out DMA is split across the
    sync and ACT HWDGE queues (64-partition DMAs run at half rate).

Hard-won hazard notes (HW-measured, this container):
  - One DMA per semaphore threshold.  N DMAs bumping one semaphore by 16
    each with consumers waiting partial thresholds (16*(j+1)) is UNSOUND:
    the 16 SDMA engines complete their per-DMA slices with skew, so
    engines running ahead on later DMAs push the count past 16*(j+1)
    while a laggard is still landing chunk j -> torn reads, sticky
    per-NEFF-load nondeterministic corruption.
  - A DVE memset does not touch PSUM has_written bits and engine start is
    skewed by iram loads, so memset-then-accumulate races the first
    matmuls.  start=True on each bank's first matmul replaces it safely.
  - Engine drains (fusable=False) before the incs that release the PSUM
    evacuation and the final out-DMA: those consumers read within ns of
    the inc and the producer's writes land after instruction commit.
"""

import numpy as np

DECAY = 0.95
B, S, H, Dk, Dv = 4, 4096, 16, 64, 64
N_CORES = 8
NS = (B * S) // N_CORES          # 2048 rows per core
P = 128                          # partitions
CHUNKS = NS // P                 # 16 contraction chunks of 128 rows
MEGA = 4                         # chunks per buffer slot group
N_MEGA = CHUNKS // MEGA          # 4 slot groups, double-buffered
FD = H * Dk                      # 1024 features per row

_nc_cache = None


def _build_nc():
    from contextlib import ExitStack

    import concourse.bass as bass
    import concourse.mybir as mybir

    f32 = mybir.dt.float32
    bf16 = mybir.dt.bfloat16
    nc = bass.Bass()

    keys_d = nc.dram_tensor("keys", (NS, FD), f32, kind="ExternalInput")
    vals_d = nc.dram_tensor("values", (NS, FD), f32, kind="ExternalInput")
    rho_d = nc.dram_tensor("rho", (P, P), f32, kind="ExternalInput")
    out_d = nc.dram_tensor("delta", (Dk, H * Dv), f32, kind="ExternalOutput")

    # Raw bass (no Tile): this container's walrus rejects engine
    # instructions carrying >1 attached semaphore wait, so all waits are
    # standalone sequencer wait_ge ops and every hazard is hand-managed.
    #
    # Chunk c = m*4+j covers token rows [c*128, (c+1)*128).  kt/vt/ktb/vtb
    # are double-buffered over m (slot m%2), 4 chunks per slot.
    #
    # Semaphore roles:
    #   kcs[c]/vcs[c]: one per chunk DMA, +16 at completion (sound: one
    #     DMA per threshold, see module docstring)
    #   scale_sem: +1 per DVE key-scale  -> c+1 after scale of chunk c
    #   act_sem:   +1 per ACT value-cast -> c+1 after cast of chunk c
    #   pe_sem:    +1 per matmul group   -> c+1 after group of chunk c
    #   evac_sem:  2 after DVE evacuates PSUM to out_t
    #   out_sem:   +16 per out-DMA half (sync + scalar queues) -> 32 done
    with ExitStack() as ctx:
        kt = [
            ctx.enter_context(nc.sbuf_tensor(f"kt{i}", [P, MEGA, FD], f32))
            for i in range(2)
        ]
        vt = [
            ctx.enter_context(nc.sbuf_tensor(f"vt{i}", [P, MEGA, FD], f32))
            for i in range(2)
        ]
        ktb = [
            ctx.enter_context(nc.sbuf_tensor(f"ktb{i}", [P, MEGA, FD], bf16))
            for i in range(2)
        ]
        vtb = [
            ctx.enter_context(nc.sbuf_tensor(f"vtb{i}", [P, MEGA, FD], bf16))
            for i in range(2)
        ]
        rho_t = ctx.enter_context(nc.sbuf_tensor("rho_t", [P, P], f32))
        warm = ctx.enter_context(nc.sbuf_tensor("warm", [Dk, 8], f32))
        out_t = ctx.enter_context(nc.sbuf_tensor("out_t", [Dk, H * Dv], f32))
        acc = [
            ctx.enter_context(nc.psum_tensor(f"acc{i}", [Dk, 8 * Dv], f32))
            for i in range(2)
        ]
        rs = ctx.enter_context(nc.semaphore(name="rs"))
        kcs = [ctx.enter_context(nc.semaphore(name=f"kc{i}")) for i in range(CHUNKS)]
        vcs = [ctx.enter_context(nc.semaphore(name=f"vc{i}")) for i in range(CHUNKS)]
        scale_sem = ctx.enter_context(nc.semaphore(name="scale_sem"))
        act_sem = ctx.enter_context(nc.semaphore(name="act_sem"))
        evac_sem = ctx.enter_context(nc.semaphore(name="evac_sem"))
        out_sem = ctx.enter_context(nc.semaphore(name="out_sem"))
        pe_sem = ctx.enter_context(nc.semaphore(name="pe_sem"))
        block = ctx.enter_context(nc.Block())

        def chunk_rows(m, j):
            c = m * MEGA + j
            return slice(c * P, (c + 1) * P)

        @block.sync
        def _(sync):
            sync.dma_start(rho_t[:], rho_d[:]).then_inc(rs, 16)
            for m in range(N_MEGA):
                for j in range(MEGA):
                    c = m * MEGA + j
                    if m >= 2:
                        # kt slot WAR: scale of chunk c-8 consumed the slot
                        sync.wait_ge(scale_sem, (m - 2) * MEGA + j + 1)
                    sync.dma_start(
                        kt[m % 2][:, j, :], keys_d[chunk_rows(m, j), :]
                    ).then_inc(kcs[c], 16)
                    if m >= 2:
                        # vt slot WAR: cast of chunk c-8 consumed the slot
                        sync.wait_ge(act_sem, (m - 2) * MEGA + j + 1)
                    sync.dma_start(
                        vt[m % 2][:, j, :], vals_d[chunk_rows(m, j), :]
                    ).then_inc(vcs[c], 16)
            sync.wait_ge(evac_sem, 1)
            sync.dma_start(out_d[:, : H * Dv // 2], out_t[:, : H * Dv // 2]).then_inc(
                out_sem, 16
            )
            sync.wait_ge(out_sem, 32)
            # all incs that will ever fire have fired: clear every
            # semaphore so the kernel is safe to run repeatedly
            # (semaphores persist across NEFF executions)
            for s in [
                rs, *kcs, *vcs, scale_sem, act_sem, evac_sem, pe_sem, out_sem,
            ]:
                sync.sem_clear(s)

        @block.vector
        def _(vector):
            # key scales by rho, f32 -> bf16
            vector.wait_ge(rs, 16)
            for m in range(N_MEGA):
                for j in range(MEGA):
                    c = m * MEGA + j
                    vector.wait_ge(kcs[c], 16)
                    if m >= 2:
                        # ktb slot WAR: matmul group of chunk c-8 done
                        vector.wait_ge(pe_sem, (m - 2) * MEGA + j + 1)
                    vector.tensor_scalar_mul(
                        ktb[m % 2][:, j, :],
                        kt[m % 2][:, j, :],
                        rho_t[:, c : c + 1],
                    ).then_inc(scale_sem, 1)
            # PSUM bank-0 evacuation (bank 1 runs in parallel on ACT)
            vector.wait_ge(pe_sem, 16)
            vector.tensor_copy(out_t[:, :512], acc[0][:])
            # drain before inc: the out DMA reads out_t right at the edge
            vector.drain(fusable=False).then_inc(evac_sem, 1)

        @block.scalar
        def _(scalar):
            # warm the activation table immediately: a lazy load at first
            # use costs ~3 us (load + auto drain) in whatever stage pays it
            scalar.copy(warm[:], warm[:])
            # value casts f32 -> bf16
            for m in range(N_MEGA):
                for j in range(MEGA):
                    c = m * MEGA + j
                    scalar.wait_ge(vcs[c], 16)
                    if m >= 2:
                        # vtb slot WAR: matmul group of chunk c-8 done
                        scalar.wait_ge(pe_sem, (m - 2) * MEGA + j + 1)
                    scalar.copy(vtb[m % 2][:, j, :], vt[m % 2][:, j, :]).then_inc(
                        act_sem, 1
                    )
            # PSUM bank-1 evacuation + its out-DMA half, all in ACT program
            # order (no cross-engine handoff needed)
            scalar.wait_ge(pe_sem, 16)
            scalar.copy(out_t[:, 512:], acc[1][:])
            scalar.drain(fusable=False)
            scalar.dma_start(
                out_d[:, H * Dv // 2 :], out_t[:, H * Dv // 2 :]
            ).then_inc(out_sem, 16)

        @block.tensor
        def _(tensor):
            for m in range(N_MEGA):
                for j in range(MEGA):
                    c = m * MEGA + j
                    tensor.wait_ge(scale_sem, c + 1)
                    tensor.wait_ge(act_sem, c + 1)
                    first = c == 0
                    last = c == CHUNKS - 1
                    for h in range(H):
                        g, hh = divmod(h, 8)
                        mm = tensor.matmul(
                            acc[g][:, hh * Dv : (hh + 1) * Dv],
                            ktb[m % 2][:, j, h * Dk : (h + 1) * Dk],
                            vtb[m % 2][:, j, h * Dv : (h + 1) * Dv],
                            # first touch of each bank clears its
                            # has_written bits; later matmuls overwrite
                            # untouched regions and accumulate touched ones
                            start=(first and hh == 0),
                            stop=last,
                            skip_group_check=True,
                        )
                        if h == H - 1 and not last:
                            # commit-attached: only WAR consumers (slot
                            # reuse) key off these counts, and commit means
                            # the matmul's SBUF reads are done.
                            mm.then_inc(pe_sem, 1)
                    if last:
                        # the 16th inc gates the PSUM evacuation: drain so
                        # the systolic array has written PSUM before DVE
                        # reads it.
                        tensor.drain(fusable=False).then_inc(pe_sem, 1)

    return nc


def _get_nc():
    global _nc_cache
    if _nc_cache is None:
        _nc_cache = _build_nc()
    return _nc_cache


def _make_in_maps(keys, values, write_strengths):
    kf = np.ascontiguousarray(keys.reshape(B * S, FD))
    vf = np.ascontiguousarray(values.reshape(B * S, FD))
    wf = np.asarray(write_strengths).reshape(B * S)
    in_maps = []
    for c in range(N_CORES):
        sl = slice(c * NS, (c + 1) * NS)
        rho_pad = np.zeros((P, P), np.float32)
        rho_pad[:, :CHUNKS] = wf[sl].reshape(CHUNKS, P).T
        in_maps.append(
            {
                "keys": np.ascontiguousarray(kf[sl]),
                "values": np.ascontiguousarray(vf[sl]),
                "rho": rho_pad,
            }
        )
    return in_maps


def _run(in_maps, **kwargs):
    from concourse.bass_utils import run_bass_kernel_spmd

    nc = _get_nc()
    return run_bass_kernel_spmd(nc, in_maps, core_ids=list(range(N_CORES)), **kwargs)


def _assemble(memory, results):
    parts = np.stack([r["delta"] for r in results], axis=0)  # (8, 64, 1024)
    delta = parts.sum(axis=0, dtype=np.float64)  # (64, 1024) in [k, h*64+v]
    delta_hkv = delta.reshape(Dk, H, Dv).transpose(1, 0, 2)  # (H, Dk, Dv)
    out = DECAY * np.asarray(memory, dtype=np.float64) + delta_hkv
    return out.astype(np.float32)


def kernel(memory, keys, values, write_strengths):
    memory = np.asarray(memory, dtype=np.float32)
    keys = np.asarray(keys, dtype=np.float32)
    values = np.asarray(values, dtype=np.float32)
    write_strengths = np.asarray(write_strengths, dtype=np.float32)

    in_maps = _make_in_maps(keys, values, write_strengths)
    res = _run(in_maps)
    return _assemble(memory, res.results)


if __name__ == "__main__":
    rng = np.random.default_rng(0)
    mem = rng.standard_normal((H, Dk, Dv), dtype=np.float32)
    k = rng.standard_normal((B, S, H, Dk), dtype=np.float32)
    v = rng.standard_normal((B, S, H, Dv), dtype=np.float32)
    w = rng.random((B, S), dtype=np.float32)
    out = kernel(mem, k, v, w)
    ref = DECAY * mem + np.einsum(
        "bs,bshk,bshv->hkv", w.astype(np.float64), k.astype(np.float64), v.astype(np.float64)
    )
    err = np.abs(out - ref).max() / np.abs(ref).max()
    print("self-check rel err:", err)
